# revision 3
# baseline (speedup 1.0000x reference)
"""PlaneAttention3D Trainium2 kernel.

Math: the three plane branches of the reference are permutations of the
token axis; multi-head attention is permutation-equivariant, so all three
branches compute the same tensor in exact arithmetic and the reference
output reduces to attn(x) + x on the identity token ordering.

Sharding: 8 cores = 2 batches x 4 query-slices (1024 tokens each).
Each core holds full K/V (all 4 heads) for its batch plus its query
slice, and produces the full [256, 1024] f32 output slice on device.
The host only slices inputs / concatenates outputs.
"""

import numpy as np

B, C = 2, 256
N = 4096          # D*H*W = 16^3
HEADS = 4
DH = 64           # head dim
NSLICES = 4       # query slices per batch
NLOC = N // NSLICES   # 1024 queries per core
NB = 512          # n-block (psum bank free size, f32)
SCALE = DH ** -0.5    # 0.125

_CACHE = {}


def _mb_groups():
    """m-block (128-wide key blocks) grouping for the exp pipeline:
    groups of 3 psum banks -> ACT reads [128, 1536] per instruction."""
    groups = []
    mb = 0
    while mb < 32:
        g = min(3, 32 - mb)
        groups.append((mb, g))
        mb += g
    return groups


def build(reps: int = 1):
    """Build + compile the SPMD program (same NEFF on all 8 cores).

    reps > 1 replicates the whole body (benchmarking only).
    """
    if reps in _CACHE:
        return _CACHE[reps]

    import concourse.bass as bass
    import concourse.tile as tile
    from concourse import bacc, mybir

    bf = mybir.dt.bfloat16
    f32 = mybir.dt.float32
    AF = mybir.ActivationFunctionType

    nc = bacc.Bacc("TRN2", target_bir_lowering=False, debug=False)

    xk_d = nc.dram_tensor("xk", [2, 128, N], bf, kind="ExternalInput")
    xq_d = nc.dram_tensor("xq", [2, 128, NLOC], bf, kind="ExternalInput")
    xr_d = nc.dram_tensor("xr", [2, 128, NLOC], f32, kind="ExternalInput")
    wq_d = nc.dram_tensor("wq", [2, 128, 256], bf, kind="ExternalInput")
    wk_d = nc.dram_tensor("wk", [2, 128, 256], bf, kind="ExternalInput")
    wv_d = nc.dram_tensor("wv", [2, 128, 256], bf, kind="ExternalInput")
    wp_d = nc.dram_tensor("wp", [4, 64, 256], bf, kind="ExternalInput")
    bp_d = nc.dram_tensor("bp", [2, 128, 1], f32, kind="ExternalInput")
    y_d = nc.dram_tensor("y", [2, 128, NLOC], f32, kind="ExternalOutput")

    with tile.TileContext(nc) as tc:
        with (
            tc.tile_pool(name="const", bufs=1) as const,
            tc.tile_pool(name="epool", bufs=2) as epool,
            tc.tile_pool(name="rpool", bufs=2) as rpool,
            tc.tile_pool(name="spsum", bufs=2, space="PSUM") as spsum,
            tc.tile_pool(name="opsum", bufs=2, space="PSUM") as opsum,
        ):
            # ---- persistent SBUF (loaded once) ----
            xk_sb = const.tile([128, 2, N], bf, tag="xk")
            xq_sb = const.tile([128, 2, NLOC], bf, tag="xq")
            xpb = const.tile([128, 2, NLOC], f32, tag="xpb")
            wq_sb = const.tile([128, 2, 256], bf, tag="wq")
            wk_sb = const.tile([128, 2, 256], bf, tag="wk")
            wv_sb = const.tile([128, 2, 256], bf, tag="wv")
            wp_sb = const.tile([64, 4, 256], bf, tag="wp")
            bp_sb = const.tile([128, 2, 1], f32, tag="bp")
            ones_t = const.tile([65, 64], f32, tag="ones")

            for kc in range(2):
                nc.sync.dma_start(out=xk_sb[:, kc, :], in_=xk_d[kc])
                nc.sync.dma_start(out=xq_sb[:, kc, :], in_=xq_d[kc])
                nc.sync.dma_start(out=wq_sb[:, kc, :], in_=wq_d[kc])
                nc.sync.dma_start(out=wk_sb[:, kc, :], in_=wk_d[kc])
                nc.sync.dma_start(out=wv_sb[:, kc, :], in_=wv_d[kc])
                nc.sync.dma_start(out=bp_sb[:, kc, :], in_=bp_d[kc])
            for h in range(HEADS):
                nc.sync.dma_start(out=wp_sb[:, h, :], in_=wp_d[h])
            nc.vector.memset(ones_t[:], 1.0)

            xr_sb = const.tile([128, 2, NLOC], f32, tag="xr")
            for kc in range(2):
                nc.sync.dma_start(out=xr_sb[:, kc, :], in_=xr_d[kc])
                # xpb = x + bp (residual + bias, folded once)
                nc.vector.tensor_scalar_add(
                    xpb[:, kc, :], xr_sb[:, kc, :], bp_sb[:, kc, :]
                )

            for rep in range(reps):
                sfx = f"_{rep}" if reps > 1 else ""
                # ---- per-rep SBUF ----
                k_sb = const.tile([128, 2, N], bf, tag="k" + sfx)
                q_sb = const.tile([128, 2, NLOC], bf, tag="q" + sfx)
                vT_sb = const.tile([128, 32, HEADS, 65], bf, tag="vT" + sfx)
                o_sbs = [
                    const.tile([64, NLOC], bf, tag=f"o{h}" + sfx,
                               name=f"osb{h}" + sfx)
                    for h in range(HEADS)
                ]
                y_sb = const.tile([128, 2, NLOC], f32, tag="y" + sfx)

                # ---- stage 1: projections ----
                # k = Wk^T.T @ x  -> [256(cout), 4096], chunk mo = head pair
                for mo in range(2):
                    for mb in range(N // NB):
                        kp = opsum.tile([128, NB], f32, tag="o")
                        for kc in range(2):
                            nc.tensor.matmul(
                                kp[:],
                                wk_sb[:, kc, mo * 128:(mo + 1) * 128],
                                xk_sb[:, kc, mb * NB:(mb + 1) * NB],
                                start=(kc == 0),
                                stop=(kc == 1),
                            )
                        nc.vector.tensor_copy(
                            k_sb[:, mo, mb * NB:(mb + 1) * NB], kp[:]
                        )
                # q on the local slice
                for mo in range(2):
                    for nb in range(NLOC // NB):
                        qp = opsum.tile([128, NB], f32, tag="o")
                        for kc in range(2):
                            nc.tensor.matmul(
                                qp[:],
                                wq_sb[:, kc, mo * 128:(mo + 1) * 128],
                                xq_sb[:, kc, nb * NB:(nb + 1) * NB],
                                start=(kc == 0),
                                stop=(kc == 1),
                            )
                        nc.vector.tensor_copy(
                            q_sb[:, mo, nb * NB:(nb + 1) * NB], qp[:]
                        )
                # vT[m, h*64+d] = x^T Wv^T ; ones column at [..., 64]
                nc.vector.memset(vT_sb[:, :, :, 64], 1.0)
                for mb in range(32):
                    vp = opsum.tile([128, 256], f32, tag="o")
                    for kc in range(2):
                        nc.tensor.matmul(
                            vp[:],
                            xk_sb[:, kc, mb * 128:(mb + 1) * 128],
                            wv_sb[:, kc, :],
                            start=(kc == 0),
                            stop=(kc == 1),
                        )
                    nc.vector.tensor_copy(
                        vT_sb[:, mb, :, 0:64],
                        vp[:].rearrange("p (h d) -> p h d", h=HEADS),
                    )

                # ---- stage 2: attention, one (head, n-block) at a time ----
                for h in range(HEADS):
                    pb = (h % 2) * 64   # partition base within chunk
                    ch = h // 2         # k/q chunk
                    for nb in range(NLOC // NB):
                        qs = q_sb[pb:pb + 64, ch, nb * NB:(nb + 1) * NB]
                        O = opsum.tile([65, NB], f32, tag="o")
                        for g0, gsz in _mb_groups():
                            S = spsum.tile([128, gsz * NB], f32, tag="s")
                            for j in range(gsz):
                                mb = g0 + j
                                nc.tensor.matmul(
                                    S[:, j * NB:(j + 1) * NB],
                                    k_sb[pb:pb + 64, ch,
                                         mb * 128:(mb + 1) * 128],
                                    qs,
                                    start=True,
                                    stop=True,
                                )
                            E = epool.tile([128, gsz * NB], bf, tag="e")
                            nc.scalar.activation(E[:], S[:], AF.Exp, scale=SCALE)
                            for j in range(gsz):
                                mb = g0 + j
                                nc.tensor.matmul(
                                    O[:],
                                    vT_sb[:, mb, h, :],
                                    E[:, j * NB:(j + 1) * NB],
                                    start=(mb == 0),
                                    stop=(mb == 31),
                                )
                        # normalize: O[0:64] / O[64]
                        r = rpool.tile([65, NB], f32, tag="r")
                        nc.vector.reciprocal(r[64:65, :], O[64:65, :])
                        Bp = opsum.tile([64, NB], f32, tag="o")
                        nc.tensor.matmul(
                            Bp[:], ones_t[64:65, :], r[64:65, :],
                            start=True, stop=True,
                        )
                        bsb = rpool.tile([64, NB], f32, tag="b")
                        nc.vector.tensor_copy(bsb[:], Bp[:])
                        nc.vector.tensor_mul(
                            o_sbs[h][:, nb * NB:(nb + 1) * NB],
                            O[0:64, :],
                            bsb[:],
                        )

                # ---- stage 3: output projection + residual ----
                for mo in range(2):
                    for nb in range(NLOC // NB):
                        P = opsum.tile([128, NB], f32, tag="o")
                        for h in range(HEADS):
                            nc.tensor.matmul(
                                P[:],
                                wp_sb[:, h, mo * 128:(mo + 1) * 128],
                                o_sbs[h][:, nb * NB:(nb + 1) * NB],
                                start=(h == 0),
                                stop=(h == HEADS - 1),
                            )
                        nc.vector.tensor_add(
                            y_sb[:, mo, nb * NB:(nb + 1) * NB],
                            P[:],
                            xpb[:, mo, nb * NB:(nb + 1) * NB],
                        )
                for mo in range(2):
                    nc.sync.dma_start(out=y_d[mo], in_=y_sb[:, mo, :])

    nc.compile()
    _CACHE[reps] = nc
    return nc


def make_in_maps(x, Wqkv, Wp, bp):
    import ml_dtypes

    bf16 = ml_dtypes.bfloat16
    x2 = np.ascontiguousarray(x.reshape(B, C, N))
    wqT = np.ascontiguousarray(Wqkv[0:256].T).astype(bf16).reshape(2, 128, 256)
    wkT = np.ascontiguousarray(Wqkv[256:512].T).astype(bf16).reshape(2, 128, 256)
    wvT = np.ascontiguousarray(Wqkv[512:768].T).astype(bf16).reshape(2, 128, 256)
    wpT = np.ascontiguousarray(Wp.T).astype(bf16).reshape(4, 64, 256)
    bp2 = np.ascontiguousarray(bp.astype(np.float32)).reshape(2, 128, 1)

    in_maps = []
    for core in range(8):
        b, s = divmod(core, NSLICES)
        xb = x2[b]
        sl = slice(s * NLOC, (s + 1) * NLOC)
        in_maps.append({
            "xk": np.ascontiguousarray(xb).astype(bf16).reshape(2, 128, N),
            "xq": np.ascontiguousarray(xb[:, sl]).astype(bf16).reshape(2, 128, NLOC),
            "xr": np.ascontiguousarray(xb[:, sl]).astype(np.float32).reshape(2, 128, NLOC),
            "wq": wqT, "wk": wkT, "wv": wvT, "wp": wpT, "bp": bp2,
        })
    return in_maps


def gather(results, x):
    out = np.empty((B, C, N), dtype=np.float32)
    for core in range(8):
        b, s = divmod(core, NSLICES)
        out[b, :, s * NLOC:(s + 1) * NLOC] = results[core]["y"].reshape(C, NLOC)
    return out.reshape(x.shape)


def kernel(x, Wqkv, Wp, bp):
    from concourse.bass_utils import run_bass_kernel_spmd

    nc = build()
    in_maps = make_in_maps(np.asarray(x), np.asarray(Wqkv),
                           np.asarray(Wp), np.asarray(bp))
    res = run_bass_kernel_spmd(nc, in_maps, core_ids=list(range(8)))
    return gather(res.results, np.asarray(x))


# revision 10
# speedup vs baseline: 1.1285x; 1.1285x over previous
"""PlaneAttention3D Trainium2 kernel.

Math: the three plane branches of the reference are permutations of the
token axis; multi-head attention is permutation-equivariant, so all three
branches compute the same tensor in exact arithmetic and the reference
output reduces to attn(x) + x on the identity token ordering.

Sharding: 8 cores = 2 batches x 4 query-slices (1024 tokens each).
Each core holds full K/V (all 4 heads) for its batch plus its query
slice, and produces the full [256, 1024] f32 output slice on device.
The host only slices/rolls inputs and concatenates outputs.

Trick: the host rolls the key/value token axis per core so the core's
query slice is always columns [0, 1024) of its xk input — attention is
invariant to a consistent permutation of the key axis, and this makes
the program identical on all cores (pure SPMD, no partition id).
"""

import numpy as np

B, C = 2, 256
N = 4096          # D*H*W = 16^3
HEADS = 4
DH = 64           # head dim
NSLICES = 4       # query slices per batch
NLOC = N // NSLICES   # 1024 queries per core
NB = 512          # n-block (psum bank free size, f32)
SCALE = DH ** -0.5    # 0.125

_CACHE = {}


GSZ = 2  # m-blocks (128-wide) per exp group; group = GSZ psum banks


def _mb_groups():
    """m-block grouping for the exp pipeline: ACT reads [128, GSZ*512]."""
    groups = []
    mb = 0
    while mb < 32:
        g = min(GSZ, 32 - mb)
        groups.append((mb, g))
        mb += g
    return groups


def build(reps: int = 1):
    """Build + compile the SPMD program (same NEFF on all 8 cores).

    reps > 1 replicates the whole body (benchmarking only).
    """
    if reps in _CACHE:
        return _CACHE[reps]

    import concourse.bass as bass
    import concourse.tile as tile
    from concourse import bacc, mybir

    bf = mybir.dt.bfloat16
    f32 = mybir.dt.float32
    AF = mybir.ActivationFunctionType

    nc = bacc.Bacc("TRN2", target_bir_lowering=False, debug=False)

    xk_d = nc.dram_tensor("xk", [2, 128, N], bf, kind="ExternalInput")
    xr_d = nc.dram_tensor("xr", [2, 128, NLOC], f32, kind="ExternalInput")
    wq_d = nc.dram_tensor("wq", [2, 128, 256], bf, kind="ExternalInput")
    wk_d = nc.dram_tensor("wk", [2, 128, 256], bf, kind="ExternalInput")
    wv_d = nc.dram_tensor("wv", [2, 128, 256], bf, kind="ExternalInput")
    wp_d = nc.dram_tensor("wp", [4, 64, 256], bf, kind="ExternalInput")
    bp_d = nc.dram_tensor("bp", [2, 128, 1], f32, kind="ExternalInput")
    y_d = nc.dram_tensor("y", [2, 128, NLOC], f32, kind="ExternalOutput")

    with tile.TileContext(nc) as tc:
        with (
            tc.tile_pool(name="const", bufs=1) as const,
            tc.tile_pool(name="epool", bufs=4) as epool,
            tc.tile_pool(name="rpool", bufs=2) as rpool,
            tc.tile_pool(name="spsum", bufs=2, space="PSUM") as spsum,
            tc.tile_pool(name="opsum", bufs=4, space="PSUM") as opsum,
        ):
            # ---- persistent SBUF ----
            xk_sb = const.tile([128, 2, N], bf, tag="xk")
            xpb = const.tile([128, 2, NLOC], f32, tag="xpb")
            wq_sb = const.tile([128, 2, 256], bf, tag="wq")
            wk_sb = const.tile([128, 2, 256], bf, tag="wk")
            wv_sb = const.tile([128, 2, 256], bf, tag="wv")
            wp_sb = const.tile([64, 4, 256], bf, tag="wp")
            bp_sb = const.tile([128, 2, 1], f32, tag="bp")
            ones_t = const.tile([65, 64], bf, tag="ones")
            scr = const.tile([1, 64], f32, tag="scr")

            # weights first (small, gate everything)
            for kc in range(2):
                nc.sync.dma_start(out=wq_sb[:, kc, :], in_=wq_d[kc])
                nc.sync.dma_start(out=wk_sb[:, kc, :], in_=wk_d[kc])
            nc.vector.memset(ones_t[:], 1.0)
            # dummy exp: pull the ACT table load into the DMA phase
            nc.scalar.activation(scr[:], ones_t[0:1, :], AF.Exp, scale=1.0)

            # xk streamed in 512-column blocks (first block gates q-proj)
            for cb in range(8):
                sl = slice(cb * 512, (cb + 1) * 512)
                for kc in range(2):
                    nc.sync.dma_start(out=xk_sb[:, kc, sl], in_=xk_d[kc, :, sl])
                if cb == 0:
                    for kc in range(2):
                        nc.sync.dma_start(out=wv_sb[:, kc, :], in_=wv_d[kc])
                if cb == 1:
                    for kc in range(2):
                        nc.sync.dma_start(out=bp_sb[:, kc, :], in_=bp_d[kc])
                    for h in range(HEADS):
                        nc.sync.dma_start(out=wp_sb[:, h, :], in_=wp_d[h])

            # ---- per-rep body ----
            for rep in range(reps):
                sfx = f"_{rep}" if reps > 1 else ""
                k_sb = const.tile([128, 2, N], bf, tag="k" + sfx, name="ksb" + sfx)
                q_sb = const.tile([128, 2, NLOC], bf, tag="q" + sfx,
                                  name="qsb" + sfx)
                vT_sb = const.tile([128, 32, HEADS, 65], bf, tag="vT" + sfx,
                                   name="vTsb" + sfx)
                o_sbs = [
                    const.tile([64, NLOC], bf, tag=f"o{h}" + sfx,
                               name=f"osb{h}" + sfx)
                    for h in range(HEADS)
                ]
                y_sb = const.tile([128, 2, NLOC], f32, tag="y" + sfx,
                                  name="ysb" + sfx)

                def attn_group(h, nb, O, g0, gsz):
                    """S^T matmuls + exp + AV accumulate for one mb-group."""
                    pb = (h % 2) * 64
                    ch = h // 2
                    qs = q_sb[pb:pb + 64, ch, nb * NB:(nb + 1) * NB]
                    S = spsum.tile([128, gsz * NB], f32, tag="s", name="Sps")
                    for j in range(gsz):
                        mb = g0 + j
                        nc.tensor.matmul(
                            S[:, j * NB:(j + 1) * NB],
                            k_sb[pb:pb + 64, ch, mb * 128:(mb + 1) * 128],
                            qs,
                            start=True,
                            stop=True,
                        )
                    E = epool.tile([128, gsz * NB], bf, tag="e", name="E")
                    nc.scalar.activation(E[:], S[:], AF.Exp, scale=SCALE)
                    for j in range(gsz):
                        mb = g0 + j
                        nc.tensor.matmul(
                            O[:],
                            vT_sb[:, mb, h, :],
                            E[:, j * NB:(j + 1) * NB],
                            start=(mb == 0),
                            stop=(mb == 31),
                        )

                def attn_finish(h, nb, O):
                    """normalize O[0:64] by O[64] into o_sbs[h]."""
                    r = rpool.tile([65, NB], bf, tag="r", name="r")
                    with nc.allow_low_precision(
                        "softmax recip in bf16; output is residual-dominated"
                    ):
                        nc.vector.reciprocal(r[64:65, :], O[64:65, :])
                    Bp = opsum.tile([64, NB], f32, tag="o", name="Bp")
                    nc.tensor.matmul(
                        Bp[:], ones_t[64:65, :], r[64:65, :],
                        start=True, stop=True,
                    )
                    bsb = rpool.tile([64, NB], f32, tag="b", name="bsb")
                    nc.vector.tensor_copy(bsb[:], Bp[:])
                    nc.vector.tensor_mul(
                        o_sbs[h][:, nb * NB:(nb + 1) * NB],
                        O[0:64, :],
                        bsb[:],
                    )

                # q projection (only needs xk columns 0:1024); copies on ACT,
                # which is otherwise idle until the first exp
                for mo in range(2):
                    for nb in range(NLOC // NB):
                        qp = opsum.tile([128, NB], f32, tag="o", name="qp")
                        for kc in range(2):
                            nc.tensor.matmul(
                                qp[:],
                                wq_sb[:, kc, mo * 128:(mo + 1) * 128],
                                xk_sb[:, kc, nb * NB:(nb + 1) * NB],
                                start=(kc == 0),
                                stop=(kc == 1),
                            )
                        if nb == 0:
                            nc.scalar.copy(
                                q_sb[:, mo, nb * NB:(nb + 1) * NB], qp[:])
                        else:
                            nc.vector.tensor_copy(
                                q_sb[:, mo, nb * NB:(nb + 1) * NB], qp[:])

                def kproj_group(mo, g0, gsz):
                    sl = slice(g0 * 128, (g0 + gsz) * 128)
                    kp = opsum.tile([128, gsz * 128], f32, tag="o", name="kp")
                    for kc in range(2):
                        nc.tensor.matmul(
                            kp[:],
                            wk_sb[:, kc, mo * 128:(mo + 1) * 128],
                            xk_sb[:, kc, sl],
                            start=(kc == 0),
                            stop=(kc == 1),
                        )
                    nc.vector.tensor_copy(k_sb[:, mo, sl], kp[:])

                def vproj_group(g0, gsz):
                    for j in range(gsz):
                        mb = g0 + j
                        vp = opsum.tile([128, 256], f32, tag="o", name="vp")
                        for kc in range(2):
                            nc.tensor.matmul(
                                vp[:],
                                xk_sb[:, kc, mb * 128:(mb + 1) * 128],
                                wv_sb[:, kc, :],
                                start=(kc == 0),
                                stop=(kc == 1),
                            )
                        nc.vector.tensor_copy(
                            vT_sb[:, mb, :, 0:64],
                            vp[:].rearrange("p (h d) -> p h d", h=HEADS),
                        )

                # weave A: k(head-pair 0) + vT production + both h=0 passes.
                # opool budget: O00+O01 pinned + kp/vp rotating = 4 slots.
                nc.vector.memset(vT_sb[:, :, :, 64], 1.0)
                O00 = opsum.tile([65, NB], f32, tag="o", name="O00")
                O01 = opsum.tile([65, NB], f32, tag="o", name="O01")
                for g0, gsz in _mb_groups():
                    kproj_group(0, g0, gsz)
                    vproj_group(g0, gsz)
                    attn_group(0, 0, O00, g0, gsz)
                    attn_group(0, 1, O01, g0, gsz)
                attn_finish(0, 0, O00)
                attn_finish(0, 1, O01)

                # weave B: k(head-pair 1) production + both h=1 passes
                O10 = opsum.tile([65, NB], f32, tag="o", name="O10")
                O11 = opsum.tile([65, NB], f32, tag="o", name="O11")
                for g0, gsz in _mb_groups():
                    kproj_group(1, g0, gsz)
                    attn_group(1, 0, O10, g0, gsz)
                    attn_group(1, 1, O11, g0, gsz)
                attn_finish(1, 0, O10)
                attn_finish(1, 1, O11)

                # residual (+ bias) — needed only at the projection stage
                xr_sb = const.tile([128, 2, NLOC], f32, tag="xr" + sfx,
                                   name="xrsb" + sfx)
                for kc in range(2):
                    nc.sync.dma_start(out=xr_sb[:, kc, :], in_=xr_d[kc])
                    nc.vector.tensor_scalar_add(
                        xpb[:, kc, :], xr_sb[:, kc, :], bp_sb[:, kc, :]
                    )

                # ---- remaining attention (h=2,3); proj per nb right after
                for nb in range(NLOC // NB):
                    for h in (2, 3):
                        O = opsum.tile([65, NB], f32, tag="o", name="Ops")
                        for g0, gsz in _mb_groups():
                            attn_group(h, nb, O, g0, gsz)
                        attn_finish(h, nb, O)
                    # ---- projection + residual for this nb ----
                    for mo in range(2):
                        P = opsum.tile([128, NB], f32, tag="o", name="P")
                        for h in range(HEADS):
                            nc.tensor.matmul(
                                P[:],
                                wp_sb[:, h, mo * 128:(mo + 1) * 128],
                                o_sbs[h][:, nb * NB:(nb + 1) * NB],
                                start=(h == 0),
                                stop=(h == HEADS - 1),
                            )
                        nc.vector.tensor_add(
                            y_sb[:, mo, nb * NB:(nb + 1) * NB],
                            P[:],
                            xpb[:, mo, nb * NB:(nb + 1) * NB],
                        )
                        nc.sync.dma_start(
                            out=y_d[mo, :, nb * NB:(nb + 1) * NB],
                            in_=y_sb[:, mo, nb * NB:(nb + 1) * NB],
                        )

    nc.compile()
    _CACHE[reps] = nc
    return nc


def make_in_maps(x, Wqkv, Wp, bp):
    import ml_dtypes

    bf16 = ml_dtypes.bfloat16
    x2 = np.ascontiguousarray(x.reshape(B, C, N))
    wqT = np.ascontiguousarray(Wqkv[0:256].T).astype(bf16).reshape(2, 128, 256)
    wkT = np.ascontiguousarray(Wqkv[256:512].T).astype(bf16).reshape(2, 128, 256)
    wvT = np.ascontiguousarray(Wqkv[512:768].T).astype(bf16).reshape(2, 128, 256)
    wpT = np.ascontiguousarray(Wp.T).astype(bf16).reshape(4, 64, 256)
    bp2 = np.ascontiguousarray(bp.astype(np.float32)).reshape(2, 128, 1)

    in_maps = []
    for core in range(8):
        b, s = divmod(core, NSLICES)
        # roll keys so this core's query slice is always columns 0:NLOC
        xb = np.roll(x2[b], -s * NLOC, axis=1)
        in_maps.append({
            "xk": np.ascontiguousarray(xb).astype(bf16).reshape(2, 128, N),
            "xr": np.ascontiguousarray(xb[:, :NLOC]).astype(np.float32)
                    .reshape(2, 128, NLOC),
            "wq": wqT, "wk": wkT, "wv": wvT, "wp": wpT, "bp": bp2,
        })
    return in_maps


def gather(results, x):
    out = np.empty((B, C, N), dtype=np.float32)
    for core in range(8):
        b, s = divmod(core, NSLICES)
        out[b, :, s * NLOC:(s + 1) * NLOC] = results[core]["y"].reshape(C, NLOC)
    return out.reshape(x.shape)


def kernel(x, Wqkv, Wp, bp):
    from concourse.bass_utils import run_bass_kernel_spmd

    nc = build()
    in_maps = make_in_maps(np.asarray(x), np.asarray(Wqkv),
                           np.asarray(Wp), np.asarray(bp))
    res = run_bass_kernel_spmd(nc, in_maps, core_ids=list(range(8)))
    return gather(res.results, np.asarray(x))


# revision 17
# speedup vs baseline: 1.1728x; 1.0393x over previous
"""PlaneAttention3D Trainium2 kernel.

Math: the three plane branches of the reference are permutations of the
token axis; multi-head attention is permutation-equivariant, so all three
branches compute the same tensor in exact arithmetic and the reference
output reduces to attn(x) + x on the identity token ordering.

Sharding: 8 cores = 2 batches x 4 query-slices (1024 tokens each).
Each core holds full K/V (all 4 heads) for its batch plus its query
slice, and produces the full [256, 1024] f32 output slice on device.
The host only slices/rolls inputs and concatenates outputs.

Trick: the host rolls the key/value token axis per core so the core's
query slice is always columns [0, 1024) of its xk input — attention is
invariant to a consistent permutation of the key axis, and this makes
the program identical on all cores (pure SPMD, no partition id).
"""

import numpy as np

B, C = 2, 256
N = 4096          # D*H*W = 16^3
HEADS = 4
DH = 64           # head dim
NSLICES = 4       # query slices per batch
NLOC = N // NSLICES   # 1024 queries per core
NB = 512          # n-block (psum bank free size, f32)
SCALE = DH ** -0.5    # 0.125

_CACHE = {}


GSZ = 2  # m-blocks (128-wide) per exp group; group = GSZ psum banks


def _mb_groups():
    """m-block grouping for the exp pipeline: ACT reads [128, GSZ*512]."""
    groups = []
    mb = 0
    while mb < 32:
        g = min(GSZ, 32 - mb)
        groups.append((mb, g))
        mb += g
    return groups


def build(reps: int = 1):
    """Build + compile the SPMD program (same NEFF on all 8 cores).

    reps > 1 replicates the whole body (benchmarking only).
    """
    if reps in _CACHE:
        return _CACHE[reps]

    import concourse.bass as bass
    import concourse.tile as tile
    from concourse import bacc, mybir

    bf = mybir.dt.bfloat16
    f32 = mybir.dt.float32
    AF = mybir.ActivationFunctionType

    nc = bacc.Bacc("TRN2", target_bir_lowering=False, debug=False)

    xk_d = nc.dram_tensor("xk", [2, 128, N], bf, kind="ExternalInput")
    xr_d = nc.dram_tensor("xr", [2, 128, NLOC], f32, kind="ExternalInput")
    wq_d = nc.dram_tensor("wq", [2, 128, 256], bf, kind="ExternalInput")
    wk_d = nc.dram_tensor("wk", [2, 128, 256], bf, kind="ExternalInput")
    wv_d = nc.dram_tensor("wv", [2, 128, 256], bf, kind="ExternalInput")
    wp_d = nc.dram_tensor("wp", [4, 64, 256], bf, kind="ExternalInput")
    bp_d = nc.dram_tensor("bp", [2, 128, 1], f32, kind="ExternalInput")
    y_d = nc.dram_tensor("y", [2, 128, NLOC], f32, kind="ExternalOutput")

    with tile.TileContext(nc) as tc:
        with (
            tc.tile_pool(name="const", bufs=1) as const,
            tc.tile_pool(name="epool", bufs=4) as epool,
            tc.tile_pool(name="rpool", bufs=2) as rpool,
            tc.tile_pool(name="spsum", bufs=2, space="PSUM") as spsum,
            tc.tile_pool(name="opsum", bufs=4, space="PSUM") as opsum,
        ):
            # ---- persistent SBUF ----
            xk_sb = const.tile([128, 2, N], bf, tag="xk")
            xpb = const.tile([128, 2, NLOC], f32, tag="xpb")
            wq_sb = const.tile([128, 2, 256], bf, tag="wq")
            wk_sb = const.tile([128, 2, 256], bf, tag="wk")
            wv_sb = const.tile([128, 2, 256], bf, tag="wv")
            wp_sb = const.tile([64, 4, 256], bf, tag="wp")
            bp_sb = const.tile([128, 2, 1], f32, tag="bp")
            ones_t = const.tile([65, 64], bf, tag="ones")
            scr = const.tile([1, 64], f32, tag="scr")

            # weights first (small, gate everything)
            for kc in range(2):
                nc.gpsimd.dma_start(out=wq_sb[:, kc, :], in_=wq_d[kc])
                nc.gpsimd.dma_start(out=wk_sb[:, kc, :], in_=wk_d[kc])
            nc.vector.memset(ones_t[:], 1.0)
            # dummy exp: pull the ACT table load into the DMA phase
            nc.scalar.activation(scr[:], ones_t[0:1, :], AF.Exp, scale=1.0)

            # xk: two small head blocks (gate q-proj / first kproj groups),
            # then one big block per chunk; misc inputs ride the Pool DGE
            for cb in range(2):
                sl = slice(cb * 512, (cb + 1) * 512)
                for kc in range(2):
                    nc.sync.dma_start(out=xk_sb[:, kc, sl], in_=xk_d[kc, :, sl])
                if cb == 0:
                    for kc in range(2):
                        nc.gpsimd.dma_start(out=wv_sb[:, kc, :], in_=wv_d[kc])
            sl = slice(1024, N)
            for kc in range(2):
                nc.sync.dma_start(out=xk_sb[:, kc, sl], in_=xk_d[kc, :, sl])
            for kc in range(2):
                nc.gpsimd.dma_start(out=bp_sb[:, kc, :], in_=bp_d[kc])
            for h in range(HEADS):
                nc.gpsimd.dma_start(out=wp_sb[:, h, :], in_=wp_d[h])

            # ---- per-rep body ----
            for rep in range(reps):
                sfx = f"_{rep}" if reps > 1 else ""
                k_sb = const.tile([128, 2, N], bf, tag="k", name="ksb" + sfx)
                q_sb = const.tile([128, 2, NLOC], bf, tag="q", name="qsb" + sfx)
                vT_sb = const.tile([128, 32, HEADS, 65], bf, tag="vT", name="vTsb" + sfx)
                o_sbs = [
                    const.tile([64, NLOC], bf, tag=f"o{h}", name=f"osb{h}" + sfx)
                    for h in range(HEADS)
                ]
                y_sb = const.tile([128, 2, NLOC], f32, tag="y", name="ysb" + sfx)

                def attn_group(h, nb, O, g0, gsz):
                    """S^T matmuls + exp + AV accumulate for one mb-group."""
                    pb = (h % 2) * 64
                    ch = h // 2
                    qs = q_sb[pb:pb + 64, ch, nb * NB:(nb + 1) * NB]
                    S = spsum.tile([128, gsz * NB], f32, tag="s", name="Sps")
                    for j in range(gsz):
                        mb = g0 + j
                        nc.tensor.matmul(
                            S[:, j * NB:(j + 1) * NB],
                            k_sb[pb:pb + 64, ch, mb * 128:(mb + 1) * 128],
                            qs,
                            start=True,
                            stop=True,
                        )
                    E = epool.tile([128, gsz * NB], bf, tag="e", name="E")
                    nc.scalar.activation(E[:], S[:], AF.Exp, scale=SCALE)
                    for j in range(gsz):
                        mb = g0 + j
                        nc.tensor.matmul(
                            O[:],
                            vT_sb[:, mb, h, :],
                            E[:, j * NB:(j + 1) * NB],
                            start=(mb == 0),
                            stop=(mb == 31),
                        )

                def attn_finish(h, nb, O):
                    """normalize O[0:64] by O[64] into o_sbs[h]."""
                    r = rpool.tile([65, NB], bf, tag="r", name="r")
                    with nc.allow_low_precision(
                        "softmax recip in bf16; output is residual-dominated"
                    ):
                        nc.vector.reciprocal(r[64:65, :], O[64:65, :])
                    Bp = opsum.tile([64, NB], f32, tag="o", name="Bp")
                    nc.tensor.matmul(
                        Bp[:], ones_t[64:65, :], r[64:65, :],
                        start=True, stop=True,
                    )
                    bsb = rpool.tile([64, NB], f32, tag="b", name="bsb")
                    nc.vector.tensor_copy(bsb[:], Bp[:])
                    nc.vector.tensor_mul(
                        o_sbs[h][:, nb * NB:(nb + 1) * NB],
                        O[0:64, :],
                        bsb[:],
                    )

                # q projection (only needs xk columns 0:1024); copies on ACT,
                # which is otherwise idle until the first exp
                for mo in range(2):
                    for nb in range(NLOC // NB):
                        qp = opsum.tile([128, NB], f32, tag="o", name="qp")
                        for kc in range(2):
                            nc.tensor.matmul(
                                qp[:],
                                wq_sb[:, kc, mo * 128:(mo + 1) * 128],
                                xk_sb[:, kc, nb * NB:(nb + 1) * NB],
                                start=(kc == 0),
                                stop=(kc == 1),
                            )
                        if nb == 0:
                            nc.scalar.copy(
                                q_sb[:, mo, nb * NB:(nb + 1) * NB], qp[:])
                        else:
                            nc.vector.tensor_copy(
                                q_sb[:, mo, nb * NB:(nb + 1) * NB], qp[:])

                def kproj_group(mo, g0, gsz):
                    sl = slice(g0 * 128, (g0 + gsz) * 128)
                    kp = opsum.tile([128, gsz * 128], f32, tag="o", name="kp")
                    for kc in range(2):
                        nc.tensor.matmul(
                            kp[:],
                            wk_sb[:, kc, mo * 128:(mo + 1) * 128],
                            xk_sb[:, kc, sl],
                            start=(kc == 0),
                            stop=(kc == 1),
                        )
                    nc.vector.tensor_copy(k_sb[:, mo, sl], kp[:])

                def vproj_group(g0, gsz):
                    for j in range(gsz):
                        mb = g0 + j
                        vp = opsum.tile([128, 256], f32, tag="o", name="vp")
                        for kc in range(2):
                            nc.tensor.matmul(
                                vp[:],
                                xk_sb[:, kc, mb * 128:(mb + 1) * 128],
                                wv_sb[:, kc, :],
                                start=(kc == 0),
                                stop=(kc == 1),
                            )
                        nc.vector.tensor_copy(
                            vT_sb[:, mb, :, 0:64],
                            vp[:].rearrange("p (h d) -> p h d", h=HEADS),
                        )

                # weave A: k(head-pair 0) + vT production + both h=0 passes.
                # opool budget: O00+O01 pinned + kp/vp rotating = 4 slots.
                # First two groups' production happens in the DMA ramp, where
                # PE is otherwise idle, so ACT starts the weave saturated.
                nc.vector.memset(vT_sb[:, :, :, 64], 1.0)
                for g0, gsz in _mb_groups()[:2]:
                    kproj_group(0, g0, gsz)
                    vproj_group(g0, gsz)
                O00 = opsum.tile([65, NB], f32, tag="o", name="O00")
                O01 = opsum.tile([65, NB], f32, tag="o", name="O01")
                for gi, (g0, gsz) in enumerate(_mb_groups()):
                    if gi >= 2:
                        kproj_group(0, g0, gsz)
                        vproj_group(g0, gsz)
                    attn_group(0, 0, O00, g0, gsz)
                    attn_group(0, 1, O01, g0, gsz)
                attn_finish(0, 0, O00)
                attn_finish(0, 1, O01)

                # weave B: k(head-pair 1) production + both h=1 passes;
                # first two k groups pre-produced to cover the transition
                for g0, gsz in _mb_groups()[:2]:
                    kproj_group(1, g0, gsz)
                O10 = opsum.tile([65, NB], f32, tag="o", name="O10")
                O11 = opsum.tile([65, NB], f32, tag="o", name="O11")
                for gi, (g0, gsz) in enumerate(_mb_groups()):
                    if gi >= 2:
                        kproj_group(1, g0, gsz)
                    attn_group(1, 0, O10, g0, gsz)
                    attn_group(1, 1, O11, g0, gsz)
                attn_finish(1, 0, O10)
                attn_finish(1, 1, O11)

                # partial projection over heads 0-1 (+residual), off the tail
                p01 = const.tile([128, 2, NLOC], f32, tag="p01", name="p01" + sfx)

                def p01_piece(nb, mo):
                    Pa = opsum.tile([128, NB], f32, tag="o", name="Pa")
                    for h in (0, 1):
                        nc.tensor.matmul(
                            Pa[:],
                            wp_sb[:, h, mo * 128:(mo + 1) * 128],
                            o_sbs[h][:, nb * NB:(nb + 1) * NB],
                            start=(h == 0),
                            stop=(h == 1),
                        )
                    nc.vector.tensor_add(
                        p01[:, mo, nb * NB:(nb + 1) * NB],
                        Pa[:],
                        xpb[:, mo, nb * NB:(nb + 1) * NB],
                    )

                # residual (+ bias) — needed only at the projection stage
                xr_sb = const.tile([128, 2, NLOC], f32, tag="xr", name="xrsb" + sfx)
                for kc in range(2):
                    nc.gpsimd.dma_start(out=xr_sb[:, kc, :], in_=xr_d[kc])
                    nc.vector.tensor_scalar_add(
                        xpb[:, kc, :], xr_sb[:, kc, :], bp_sb[:, kc, :]
                    )

                def finish_half(h, nb, O, c0, cw):
                    """finish-chain for columns [c0, c0+cw) of O (tail pipelining)."""
                    r = rpool.tile([65, NB], bf, tag="r", name="rh")
                    with nc.allow_low_precision(
                        "softmax recip in bf16; output is residual-dominated"
                    ):
                        nc.vector.reciprocal(r[64:65, c0:c0 + cw],
                                             O[64:65, c0:c0 + cw])
                    Bp = opsum.tile([64, NB], f32, tag="o", name="Bph")
                    nc.tensor.matmul(
                        Bp[:, 0:cw], ones_t[64:65, :], r[64:65, c0:c0 + cw],
                        start=True, stop=True,
                    )
                    bsb = rpool.tile([64, NB], f32, tag="b", name="bsbh")
                    nc.scalar.copy(bsb[:, 0:cw], Bp[:, 0:cw])
                    nc.vector.tensor_mul(
                        o_sbs[h][:, nb * NB + c0:nb * NB + c0 + cw],
                        O[0:64, c0:c0 + cw],
                        bsb[:, 0:cw],
                    )

                def proj_tail(nb, mo, c0, cw):
                    base = nb * NB + c0
                    P = opsum.tile([128, NB], f32, tag="o", name="P")
                    for h in (2, 3):
                        nc.tensor.matmul(
                            P[:, 0:cw],
                            wp_sb[:, h, mo * 128:(mo + 1) * 128],
                            o_sbs[h][:, base:base + cw],
                            start=(h == 2),
                            stop=(h == 3),
                        )
                    nc.vector.tensor_add(
                        y_sb[:, mo, base:base + cw],
                        P[:, 0:cw],
                        p01[:, mo, base:base + cw],
                    )
                    # spread output DMAs across idle DGE queues
                    engs = {(0, 0): nc.sync, (1, 0): nc.gpsimd,
                            (0, 1): nc.scalar, (1, 1): nc.sync}
                    eng = engs[(mo, 1 if c0 else 0)] if cw < NB else (
                        nc.sync if mo == 0 else nc.gpsimd)
                    eng.dma_start(
                        out=y_d[mo, :, base:base + cw],
                        in_=y_sb[:, mo, base:base + cw],
                    )

                # ---- remaining attention (h=2,3); proj per nb right after
                for nb in range(NLOC // NB):
                    last = (nb == NLOC // NB - 1)
                    for h in (2, 3):
                        O = opsum.tile([65, NB], f32, tag="o", name="Ops")
                        for gi, (g0, gsz) in enumerate(_mb_groups()):
                            attn_group(h, nb, O, g0, gsz)
                            if h == 2 and gi in (5, 10):
                                p01_piece(nb, gi // 8)
                        if h == 3 and last:
                            # pipeline the tail in two half-width chains
                            for c0 in (0, NB // 2):
                                finish_half(h, nb, O, c0, NB // 2)
                                for mo in range(2):
                                    proj_tail(nb, mo, c0, NB // 2)
                        else:
                            attn_finish(h, nb, O)
                    if not last:
                        for mo in range(2):
                            proj_tail(nb, mo, 0, NB)

    nc.compile()
    _CACHE[reps] = nc
    return nc


def make_in_maps(x, Wqkv, Wp, bp):
    import ml_dtypes

    bf16 = ml_dtypes.bfloat16
    x2 = np.ascontiguousarray(x.reshape(B, C, N))
    wqT = np.ascontiguousarray(Wqkv[0:256].T).astype(bf16).reshape(2, 128, 256)
    wkT = np.ascontiguousarray(Wqkv[256:512].T).astype(bf16).reshape(2, 128, 256)
    wvT = np.ascontiguousarray(Wqkv[512:768].T).astype(bf16).reshape(2, 128, 256)
    wpT = np.ascontiguousarray(Wp.T).astype(bf16).reshape(4, 64, 256)
    bp2 = np.ascontiguousarray(bp.astype(np.float32)).reshape(2, 128, 1)

    in_maps = []
    for core in range(8):
        b, s = divmod(core, NSLICES)
        # roll keys so this core's query slice is always columns 0:NLOC
        xb = np.roll(x2[b], -s * NLOC, axis=1)
        in_maps.append({
            "xk": np.ascontiguousarray(xb).astype(bf16).reshape(2, 128, N),
            "xr": np.ascontiguousarray(xb[:, :NLOC]).astype(np.float32)
                    .reshape(2, 128, NLOC),
            "wq": wqT, "wk": wkT, "wv": wvT, "wp": wpT, "bp": bp2,
        })
    return in_maps


def gather(results, x):
    out = np.empty((B, C, N), dtype=np.float32)
    for core in range(8):
        b, s = divmod(core, NSLICES)
        out[b, :, s * NLOC:(s + 1) * NLOC] = results[core]["y"].reshape(C, NLOC)
    return out.reshape(x.shape)


def kernel(x, Wqkv, Wp, bp):
    from concourse.bass_utils import run_bass_kernel_spmd

    nc = build()
    in_maps = make_in_maps(np.asarray(x), np.asarray(Wqkv),
                           np.asarray(Wp), np.asarray(bp))
    res = run_bass_kernel_spmd(nc, in_maps, core_ids=list(range(8)))
    return gather(res.results, np.asarray(x))


# revision 25
# speedup vs baseline: 1.1853x; 1.0106x over previous
"""PlaneAttention3D Trainium2 kernel.

Math: the three plane branches of the reference are permutations of the
token axis; multi-head attention is permutation-equivariant, so all three
branches compute the same tensor in exact arithmetic and the reference
output reduces to attn(x) + x on the identity token ordering.

Sharding: 8 cores = 2 batches x 4 query-slices (1024 tokens each).
Each core holds full K/V (all 4 heads) for its batch plus its query
slice, and produces the full [256, 1024] f32 output slice on device.
The host only slices/rolls inputs and concatenates outputs.

Trick: the host rolls the key/value token axis per core so the core's
query slice is always columns [0, 1024) of its xk input — attention is
invariant to a consistent permutation of the key axis, and this makes
the program identical on all cores (pure SPMD, no partition id).
"""

import numpy as np

B, C = 2, 256
N = 4096          # D*H*W = 16^3
HEADS = 4
DH = 64           # head dim
NSLICES = 4       # query slices per batch
NLOC = N // NSLICES   # 1024 queries per core
NB = 512          # n-block (psum bank free size, f32)
SCALE = DH ** -0.5    # 0.125

_CACHE = {}


GSZ = 2  # m-blocks (128-wide) per exp group; group = GSZ psum banks


def _mb_groups():
    """m-block grouping for the exp pipeline: ACT reads [128, GSZ*512]."""
    groups = []
    mb = 0
    while mb < 32:
        g = min(GSZ, 32 - mb)
        groups.append((mb, g))
        mb += g
    return groups


def build(reps: int = 1):
    """Build + compile the SPMD program (same NEFF on all 8 cores).

    reps > 1 replicates the whole body (benchmarking only).
    """
    if reps in _CACHE:
        return _CACHE[reps]

    import concourse.bass as bass
    import concourse.tile as tile
    from concourse import bacc, mybir

    bf = mybir.dt.bfloat16
    f32 = mybir.dt.float32
    AF = mybir.ActivationFunctionType

    nc = bacc.Bacc("TRN2", target_bir_lowering=False, debug=False)

    xk_d = nc.dram_tensor("xk", [2, 128, N], bf, kind="ExternalInput")
    xr_d = nc.dram_tensor("xr", [2, 128, NLOC], f32, kind="ExternalInput")
    wq_d = nc.dram_tensor("wq", [2, 128, 256], bf, kind="ExternalInput")
    wk_d = nc.dram_tensor("wk", [2, 128, 256], bf, kind="ExternalInput")
    wv_d = nc.dram_tensor("wv", [2, 128, 256], bf, kind="ExternalInput")
    wp_d = nc.dram_tensor("wp", [4, 64, 256], bf, kind="ExternalInput")
    bp_d = nc.dram_tensor("bp", [2, 128, 1], f32, kind="ExternalInput")
    y_d = nc.dram_tensor("y", [2, 128, NLOC], f32, kind="ExternalOutput")

    with tile.TileContext(nc) as tc:
        with (
            tc.tile_pool(name="const", bufs=1) as const,
            tc.tile_pool(name="epool", bufs=6) as epool,
            tc.tile_pool(name="rpool", bufs=4) as rpool,
            tc.tile_pool(name="spsum", bufs=2, space="PSUM") as spsum,
            tc.tile_pool(name="opsum", bufs=4, space="PSUM") as opsum,
        ):
            # ---- persistent SBUF ----
            xk_sb = const.tile([128, 2, N], bf, tag="xk")
            xpb = const.tile([128, 2, NLOC], f32, tag="xpb")
            wq_sb = const.tile([128, 2, 256], bf, tag="wq")
            wk_sb = const.tile([128, 2, 256], bf, tag="wk")
            wv_sb = const.tile([128, 2, 256], bf, tag="wv")
            wp_sb = const.tile([64, 4, 256], bf, tag="wp")
            bp_sb = const.tile([128, 2, 1], f32, tag="bp")
            ones_t = const.tile([65, 64], bf, tag="ones")
            scr = const.tile([1, 64], f32, tag="scr")

            # weights first (small, gate everything)
            for kc in range(2):
                nc.gpsimd.dma_start(out=wq_sb[:, kc, :], in_=wq_d[kc])
                nc.gpsimd.dma_start(out=wk_sb[:, kc, :], in_=wk_d[kc])
            nc.vector.memset(ones_t[:], 1.0)
            # dummy exp: pull the ACT table load into the DMA phase
            nc.scalar.activation(scr[:], ones_t[0:1, :], AF.Exp, scale=1.0)

            # xk: two small head blocks (gate q-proj / first kproj groups),
            # then one big block per chunk; misc inputs ride the Pool DGE
            for cb in range(2):
                sl = slice(cb * 512, (cb + 1) * 512)
                for kc in range(2):
                    nc.sync.dma_start(out=xk_sb[:, kc, sl], in_=xk_d[kc, :, sl])
                if cb == 0:
                    for kc in range(2):
                        nc.gpsimd.dma_start(out=wv_sb[:, kc, :], in_=wv_d[kc])
            sl = slice(1024, N)
            for kc in range(2):
                nc.sync.dma_start(out=xk_sb[:, kc, sl], in_=xk_d[kc, :, sl])
            for kc in range(2):
                nc.gpsimd.dma_start(out=bp_sb[:, kc, :], in_=bp_d[kc])
            for h in range(HEADS):
                nc.gpsimd.dma_start(out=wp_sb[:, h, :], in_=wp_d[h])

            # ---- per-rep body ----
            for rep in range(reps):
                sfx = f"_{rep}" if reps > 1 else ""
                k_sb = const.tile([128, 2, N], bf, tag="k", name="ksb" + sfx)
                q_sb = const.tile([128, 2, NLOC], bf, tag="q", name="qsb" + sfx)
                vT_sb = const.tile([128, 32, HEADS, 65], bf, tag="vT", name="vTsb" + sfx)
                o_sbs = [
                    const.tile([64, NLOC], bf, tag=f"o{h}", name=f"osb{h}" + sfx)
                    for h in range(HEADS)
                ]
                y_sb = const.tile([128, 2, NLOC], f32, tag="y", name="ysb" + sfx)

                def attn_group(h, nb, O, g0, gsz):
                    """S^T matmuls + exp + AV accumulate for one mb-group."""
                    pb = (h % 2) * 64
                    ch = h // 2
                    qs = q_sb[pb:pb + 64, ch, nb * NB:(nb + 1) * NB]
                    S = spsum.tile([128, gsz * NB], f32, tag="s", name="Sps")
                    for j in range(gsz):
                        mb = g0 + j
                        nc.tensor.matmul(
                            S[:, j * NB:(j + 1) * NB],
                            k_sb[pb:pb + 64, ch, mb * 128:(mb + 1) * 128],
                            qs,
                            start=True,
                            stop=True,
                        )
                    E = epool.tile([128, gsz * NB], bf, tag="e", name="E")
                    nc.scalar.activation(E[:], S[:], AF.Exp, scale=SCALE)
                    for j in range(gsz):
                        mb = g0 + j
                        nc.tensor.matmul(
                            O[:],
                            vT_sb[:, mb, h, :],
                            E[:, j * NB:(j + 1) * NB],
                            start=(mb == 0),
                            stop=(mb == 31),
                        )

                def attn_group_pair(h, Oa, Ob, g0, gsz):
                    """Both nb passes of one head for one mb-group, ordered
                    S,S,exp,exp,AV,AV to avoid PE head-of-line blocking."""
                    pb = (h % 2) * 64
                    ch = h // 2
                    Ss, Es = [], []
                    for nb in range(2):
                        qs = q_sb[pb:pb + 64, ch, nb * NB:(nb + 1) * NB]
                        S = spsum.tile([128, gsz * NB], f32, tag="s",
                                       name="Spr")
                        for j in range(gsz):
                            mb = g0 + j
                            nc.tensor.matmul(
                                S[:, j * NB:(j + 1) * NB],
                                k_sb[pb:pb + 64, ch, mb * 128:(mb + 1) * 128],
                                qs,
                                start=True,
                                stop=True,
                            )
                        Ss.append(S)
                    for nb in range(2):
                        E = epool.tile([128, gsz * NB], bf, tag="e", name="Ep")
                        nc.scalar.activation(E[:], Ss[nb][:], AF.Exp,
                                             scale=SCALE)
                        Es.append(E)
                    for nb, O in ((0, Oa), (1, Ob)):
                        for j in range(gsz):
                            mb = g0 + j
                            nc.tensor.matmul(
                                O[:],
                                vT_sb[:, mb, h, :],
                                Es[nb][:, j * NB:(j + 1) * NB],
                                start=(mb == 0),
                                stop=(mb == 31),
                            )

                def attn_finish(h, nb, O):
                    """normalize O[0:64] by O[64] into o_sbs[h]."""
                    r = rpool.tile([65, NB], bf, tag="r", name="r")
                    with nc.allow_low_precision(
                        "softmax recip in bf16; output is residual-dominated"
                    ):
                        nc.vector.reciprocal(r[64:65, :], O[64:65, :])
                    Bp = opsum.tile([64, NB], f32, tag="o", name="Bp")
                    nc.tensor.matmul(
                        Bp[:], ones_t[64:65, :], r[64:65, :],
                        start=True, stop=True,
                    )
                    bsb = rpool.tile([64, NB], f32, tag="b", name="bsb")
                    nc.vector.tensor_copy(bsb[:], Bp[:])
                    nc.vector.tensor_mul(
                        o_sbs[h][:, nb * NB:(nb + 1) * NB],
                        O[0:64, :],
                        bsb[:],
                    )

                # q projection (only needs xk columns 0:1024); copies on ACT,
                # which is otherwise idle until the first exp
                for mo in range(2):
                    for nb in range(NLOC // NB):
                        qp = opsum.tile([128, NB], f32, tag="o", name="qp")
                        for kc in range(2):
                            nc.tensor.matmul(
                                qp[:],
                                wq_sb[:, kc, mo * 128:(mo + 1) * 128],
                                xk_sb[:, kc, nb * NB:(nb + 1) * NB],
                                start=(kc == 0),
                                stop=(kc == 1),
                            )
                        if nb == 0:
                            nc.scalar.copy(
                                q_sb[:, mo, nb * NB:(nb + 1) * NB], qp[:])
                        else:
                            nc.vector.tensor_copy(
                                q_sb[:, mo, nb * NB:(nb + 1) * NB], qp[:])

                def kproj_group(mo, g0, gsz):
                    sl = slice(g0 * 128, (g0 + gsz) * 128)
                    kp = opsum.tile([128, gsz * 128], f32, tag="o", name="kp")
                    for kc in range(2):
                        nc.tensor.matmul(
                            kp[:],
                            wk_sb[:, kc, mo * 128:(mo + 1) * 128],
                            xk_sb[:, kc, sl],
                            start=(kc == 0),
                            stop=(kc == 1),
                        )
                    nc.vector.tensor_copy(k_sb[:, mo, sl], kp[:])

                def vproj_group(g0, gsz):
                    for j in range(gsz):
                        mb = g0 + j
                        vp = opsum.tile([128, 256], f32, tag="o", name="vp")
                        for kc in range(2):
                            nc.tensor.matmul(
                                vp[:],
                                xk_sb[:, kc, mb * 128:(mb + 1) * 128],
                                wv_sb[:, kc, :],
                                start=(kc == 0),
                                stop=(kc == 1),
                            )
                        nc.vector.tensor_copy(
                            vT_sb[:, mb, :, 0:64],
                            vp[:].rearrange("p (h d) -> p h d", h=HEADS),
                        )

                # weave A: k(head-pair 0) + vT production + both h=0 passes.
                # opool budget: O00+O01 pinned + kp/vp rotating = 4 slots.
                # First two groups' production happens in the DMA ramp, where
                # PE is otherwise idle, so ACT starts the weave saturated.
                nc.vector.memset(vT_sb[:, :, :, 64], 1.0)
                groups = _mb_groups()
                for g0, gsz in groups[:3]:
                    kproj_group(0, g0, gsz)
                    vproj_group(g0, gsz)
                O00 = opsum.tile([65, NB], f32, tag="o", name="O00")
                O01 = opsum.tile([65, NB], f32, tag="o", name="O01")
                for gi, (g0, gsz) in enumerate(groups):
                    if gi >= 3:
                        kproj_group(0, g0, gsz)
                        vproj_group(g0, gsz)
                    attn_group_pair(0, O00, O01, g0, gsz)
                attn_finish(0, 0, O00)
                attn_finish(0, 1, O01)

                # weave B: k(head-pair 1) production + both h=1 passes;
                # first two k groups pre-produced to cover the transition
                for g0, gsz in groups[:3]:
                    kproj_group(1, g0, gsz)
                O10 = opsum.tile([65, NB], f32, tag="o", name="O10")
                O11 = opsum.tile([65, NB], f32, tag="o", name="O11")
                for gi, (g0, gsz) in enumerate(groups):
                    if gi >= 3:
                        kproj_group(1, g0, gsz)
                    attn_group_pair(1, O10, O11, g0, gsz)
                attn_finish(1, 0, O10)
                attn_finish(1, 1, O11)

                # partial projection over heads 0-1 (+residual), off the tail
                p01 = const.tile([128, 2, NLOC], f32, tag="p01", name="p01" + sfx)

                def p01_piece(nb, mo):
                    Pa = opsum.tile([128, NB], f32, tag="o", name="Pa")
                    for h in (0, 1):
                        nc.tensor.matmul(
                            Pa[:],
                            wp_sb[:, h, mo * 128:(mo + 1) * 128],
                            o_sbs[h][:, nb * NB:(nb + 1) * NB],
                            start=(h == 0),
                            stop=(h == 1),
                        )
                    nc.vector.tensor_add(
                        p01[:, mo, nb * NB:(nb + 1) * NB],
                        Pa[:],
                        xpb[:, mo, nb * NB:(nb + 1) * NB],
                    )

                # residual (+ bias) — needed only at the projection stage
                xr_sb = const.tile([128, 2, NLOC], f32, tag="xr", name="xrsb" + sfx)
                for kc in range(2):
                    nc.gpsimd.dma_start(out=xr_sb[:, kc, :], in_=xr_d[kc])
                    nc.vector.tensor_scalar_add(
                        xpb[:, kc, :], xr_sb[:, kc, :], bp_sb[:, kc, :]
                    )

                def finish_half(h, nb, O, c0, cw):
                    """finish-chain for columns [c0, c0+cw) of O (tail pipelining)."""
                    r = rpool.tile([65, NB], bf, tag="r", name="rh")
                    with nc.allow_low_precision(
                        "softmax recip in bf16; output is residual-dominated"
                    ):
                        nc.vector.reciprocal(r[64:65, c0:c0 + cw],
                                             O[64:65, c0:c0 + cw])
                    Bp = opsum.tile([64, NB], f32, tag="o", name="Bph")
                    nc.tensor.matmul(
                        Bp[:, 0:cw], ones_t[64:65, :], r[64:65, c0:c0 + cw],
                        start=True, stop=True,
                    )
                    bsb = rpool.tile([64, NB], f32, tag="b", name="bsbh")
                    nc.scalar.copy(bsb[:, 0:cw], Bp[:, 0:cw])
                    nc.vector.tensor_mul(
                        o_sbs[h][:, nb * NB + c0:nb * NB + c0 + cw],
                        O[0:64, c0:c0 + cw],
                        bsb[:, 0:cw],
                    )

                def proj_tail(nb, mo, c0, cw):
                    base = nb * NB + c0
                    P = opsum.tile([128, NB], f32, tag="o", name="P")
                    for h in (2, 3):
                        nc.tensor.matmul(
                            P[:, 0:cw],
                            wp_sb[:, h, mo * 128:(mo + 1) * 128],
                            o_sbs[h][:, base:base + cw],
                            start=(h == 2),
                            stop=(h == 3),
                        )
                    nc.vector.tensor_add(
                        y_sb[:, mo, base:base + cw],
                        P[:, 0:cw],
                        p01[:, mo, base:base + cw],
                    )
                    # spread output DMAs across idle DGE queues
                    engs = {(0, 0): nc.sync, (1, 0): nc.gpsimd,
                            (0, 1): nc.scalar, (1, 1): nc.sync}
                    eng = engs[(mo, 1 if c0 else 0)] if cw < NB else (
                        nc.sync if mo == 0 else nc.gpsimd)
                    eng.dma_start(
                        out=y_d[mo, :, base:base + cw],
                        in_=y_sb[:, mo, base:base + cw],
                    )

                # ---- remaining attention (h=2,3); proj per nb right after
                for nb in range(NLOC // NB):
                    last = (nb == NLOC // NB - 1)
                    for h in (2, 3):
                        O = opsum.tile([65, NB], f32, tag="o", name="Ops")
                        for gi, (g0, gsz) in enumerate(_mb_groups()):
                            attn_group(h, nb, O, g0, gsz)
                            if h == 2 and gi in (5, 10):
                                p01_piece(nb, gi // 8)
                        if h == 3 and last:
                            # pipeline the tail in two half-width chains
                            for c0 in (0, NB // 2):
                                finish_half(h, nb, O, c0, NB // 2)
                                for mo in range(2):
                                    proj_tail(nb, mo, c0, NB // 2)
                        else:
                            attn_finish(h, nb, O)
                    if not last:
                        for mo in range(2):
                            proj_tail(nb, mo, 0, NB)

    nc.compile()
    _CACHE[reps] = nc
    return nc


def make_in_maps(x, Wqkv, Wp, bp):
    import ml_dtypes

    bf16 = ml_dtypes.bfloat16
    x2 = np.ascontiguousarray(x.reshape(B, C, N))
    wqT = np.ascontiguousarray(Wqkv[0:256].T).astype(bf16).reshape(2, 128, 256)
    wkT = np.ascontiguousarray(Wqkv[256:512].T).astype(bf16).reshape(2, 128, 256)
    wvT = np.ascontiguousarray(Wqkv[512:768].T).astype(bf16).reshape(2, 128, 256)
    wpT = np.ascontiguousarray(Wp.T).astype(bf16).reshape(4, 64, 256)
    bp2 = np.ascontiguousarray(bp.astype(np.float32)).reshape(2, 128, 1)

    in_maps = []
    for core in range(8):
        b, s = divmod(core, NSLICES)
        # roll keys so this core's query slice is always columns 0:NLOC
        xb = np.roll(x2[b], -s * NLOC, axis=1)
        in_maps.append({
            "xk": np.ascontiguousarray(xb).astype(bf16).reshape(2, 128, N),
            "xr": np.ascontiguousarray(xb[:, :NLOC]).astype(np.float32)
                    .reshape(2, 128, NLOC),
            "wq": wqT, "wk": wkT, "wv": wvT, "wp": wpT, "bp": bp2,
        })
    return in_maps


def gather(results, x):
    out = np.empty((B, C, N), dtype=np.float32)
    for core in range(8):
        b, s = divmod(core, NSLICES)
        out[b, :, s * NLOC:(s + 1) * NLOC] = results[core]["y"].reshape(C, NLOC)
    return out.reshape(x.shape)


def kernel(x, Wqkv, Wp, bp):
    from concourse.bass_utils import run_bass_kernel_spmd

    nc = build()
    in_maps = make_in_maps(np.asarray(x), np.asarray(Wqkv),
                           np.asarray(Wp), np.asarray(bp))
    res = run_bass_kernel_spmd(nc, in_maps, core_ids=list(range(8)))
    return gather(res.results, np.asarray(x))
